# revision 12
# baseline (speedup 1.0000x reference)
"""Trainium2 Bass kernel for nn_ChannelSegment (differential-attention MoE).

Sharding: 8 cores = 4 channels x 2 batches; core i handles (b, n) = (i//4, i%4).
Each core runs the full per-channel forward for one [L=1024, CW=512] slice.

v2 over the 311us baseline:
- Causal mask applied on the PE via an extra accumulating matmul
  (triu_k1^T @ (-300*I)) into the score PSUM bank, replacing the 78us of
  gpsimd mask multiplies. exp(scale*(s-300)) ~ 1e-21 ~ 0.
- exp batched over both attention branches in one ACT instruction
  ([128, 2, w] strided read across two PSUM banks).
- Softmax denominators broadcast across partitions with DMA
  (partition_broadcast from the PSUM accumulator row) instead of PE matmuls;
  diff combine reads the PSUM accumulators directly (no drain copies).
- Per-query diff-rms mean-squares collected into one SBUF tile by DMA and
  rsqrt'd in a single batched Ln+Exp pair (natural_log_exp set shared with
  the attention exp -> 4 activation-table loads instead of 6).
- q/k rms mean-square reduction packed into one [32, L] accumulator
  (8 j-blocks accumulate via block-select matrices), one batched rsqrt.
- v bias folded into the PSUM accumulation via a rank-1 matmul.
- Phase order keeps the PE stream dense (MM1a -> MM1b -> rms -> attention).
"""
import os
import sys

sys.path.insert(0, "/opt/trn_rl_repo")

import numpy as np
import ml_dtypes

from concourse import bacc
import concourse.tile as tile
from concourse import mybir
from concourse.bass_utils import run_bass_kernel_spmd

N_CH, CW, H, D, D2 = 4, 512, 8, 64, 32
L, B = 1024, 2
EPS = 1e-6
LAM0 = 0.2
SCALE = float(1.0 / np.sqrt(np.float32(D2)))
MASKC = -300.0

F32 = mybir.dt.float32
F32R = mybir.dt.float32r
BF16 = mybir.dt.bfloat16
AF = mybir.ActivationFunctionType
OP = mybir.AluOpType

_cache = {}


def _build():
    from contextlib import ExitStack

    nc = bacc.Bacc("TRN2", target_bir_lowering=False, num_devices=8)

    dp = nc.declare_dram_parameter
    hT_d = dp("hT", [CW, L], F32R, isOutput=False)
    wqk_d = dp("wqk", [CW, 2 * CW], F32R, isOutput=False)
    wv_d = dp("wv", [CW, CW], F32R, isOutput=False)
    wout_d = dp("wout", [CW, CW], F32R, isOutput=False)
    bqk_d = dp("bqk", [8, 128, 1], F32, isOutput=False)
    bv_d = dp("bv", [1, CW], BF16, isOutput=False)
    bout_d = dp("bout", [4, 128, 1], F32, isOutput=False)
    qmul_d = dp("qmul", [4, 128, 1], F32, isOutput=False)
    lamwhs_d = dp("lamwhs", [1, 64], F32R, isOutput=False)
    whs64_d = dp("whs64", [1, 64], F32R, isOutput=False)
    invwhsq_d = dp("invwhsq", [64, 1], BF16, isOutput=False)
    wnw_d = dp("wnw", [4, 128, 1], F32, isOutput=False)
    wrt_d = dp("wrt", [128, 1], F32, isOutput=False)
    triu_d = dp("triu", [128, 128], BF16, isOutput=False)
    neye_d = dp("neye", [128, 128], BF16, isOutput=False)
    e4all_d = dp("e4all", [128, 8, 32], BF16, isOutput=False)
    b4all_d = dp("b4all", [32, 8, 128], F32R, isOutput=False)
    selall_d = dp("selall", [97, 2, 128], F32R, isOutput=False)
    ones_d = dp("ones", [128, 1], F32R, isOutput=False)
    o1128_d = dp("o1128", [1, 128], F32R, isOutput=False)
    yT_d = dp("yT", [CW, L], F32, isOutput=True)

    with tile.TileContext(nc) as tc:
        est = ExitStack()
        est.enter_context(nc.allow_low_precision(reason="float32r/bf16 intermediates"))
        persist = est.enter_context(tc.tile_pool(name="persist", bufs=1))
        ps_mm = est.enter_context(tc.tile_pool(name="ps_mm", bufs=2, space="PSUM"))
        ps_acc = est.enter_context(tc.tile_pool(name="ps_acc", bufs=4, space="PSUM"))
        p_pool = est.enter_context(tc.tile_pool(name="p_pool", bufs=3))
        sqp = est.enter_context(tc.tile_pool(name="sqp", bufs=2))
        asqp = est.enter_context(tc.tile_pool(name="asqp", bufs=2))
        osbp = est.enter_context(tc.tile_pool(name="osbp", bufs=4))
        m1p = est.enter_context(tc.tile_pool(name="m1p", bufs=2))
        t2p = est.enter_context(tc.tile_pool(name="t2p", bufs=2))
        usqp = est.enter_context(tc.tile_pool(name="usqp", bufs=2))
        tmpp = est.enter_context(tc.tile_pool(name="tmpp", bufs=2))
        yp = est.enter_context(tc.tile_pool(name="yp", bufs=2))

        dma = nc.sync.dma_start
        dma2 = nc.scalar.dma_start

        # ---- load constants / inputs (split across the two DMA queues) ----
        wqk = [persist.tile([128, 2 * CW], F32R, tag=f"wq{k}", name=f"wq{k}") for k in range(4)]
        hT = [persist.tile([128, L], F32R, tag=f"hT{k}", name=f"hT{k}") for k in range(4)]
        for k in range(4):
            dma(out=wqk[k], in_=wqk_d[128 * k : 128 * (k + 1), :])
            dma2(out=hT[k], in_=hT_d[128 * k : 128 * (k + 1), :])
        wv = [persist.tile([128, CW], F32R, tag=f"wv{k}", name=f"wv{k}") for k in range(4)]
        wout_sb = [persist.tile([128, CW], F32R, tag=f"wo{k}", name=f"wo{k}") for k in range(4)]
        for k in range(4):
            dma(out=wv[k], in_=wv_d[128 * k : 128 * (k + 1), :])
            dma2(out=wout_sb[k], in_=wout_d[128 * k : 128 * (k + 1), :])

        bqk = [persist.tile([128, 1], F32, tag=f"bqk{j}", name=f"bqk{j}") for j in range(8)]
        for j in range(8):
            dma(out=bqk[j], in_=bqk_d[j])
        bv_bf = persist.tile([1, CW], BF16, tag="bv_bf", name="bv_bf")
        dma(out=bv_bf, in_=bv_d[:])
        bout = [persist.tile([128, 1], F32, tag=f"bout{j}", name=f"bout{j}") for j in range(4)]
        qmul = [persist.tile([128, 1], F32, tag=f"qmul{j}", name=f"qmul{j}") for j in range(4)]
        wnw = [persist.tile([128, 1], F32, tag=f"wnw{j}", name=f"wnw{j}") for j in range(4)]
        for j in range(4):
            dma2(out=bout[j], in_=bout_d[j])
            dma2(out=qmul[j], in_=qmul_d[j])
            dma2(out=wnw[j], in_=wnw_d[j])
        # stationaries for the den broadcast live at partition 64 so their
        # base partition matches the accumulator den row they stream against
        lamwhs_t = persist.tile([65, 64], F32R, tag="lamwhs", name="lamwhs")
        dma(out=lamwhs_t[64:65, :], in_=lamwhs_d[:])
        whs64_t = persist.tile([65, 64], F32R, tag="whs64", name="whs64")
        dma(out=whs64_t[64:65, :], in_=whs64_d[:])
        invwhsq = persist.tile([64, 1], BF16, tag="invwhsq", name="invwhsq")
        dma(out=invwhsq, in_=invwhsq_d[:])
        wrt = persist.tile([128, 1], F32, tag="wrt", name="wrt")
        dma(out=wrt, in_=wrt_d[:])
        triu_sb = persist.tile([128, 128], BF16, tag="triu", name="triu")
        dma(out=triu_sb, in_=triu_d[:])
        neye_sb = persist.tile([128, 128], BF16, tag="neye", name="neye")
        dma(out=neye_sb, in_=neye_d[:])
        e4all = persist.tile([128, 8, 32], BF16, tag="e4all", name="e4all")
        dma2(out=e4all, in_=e4all_d[:])
        b4all = persist.tile([32, 8, 128], F32R, tag="b4all", name="b4all")
        dma2(out=b4all, in_=b4all_d[:])
        selall = persist.tile([97, 2, 128], F32R, tag="selall", name="selall")
        dma2(out=selall, in_=selall_d[:])
        ones_r = persist.tile([128, 1], F32R, tag="ones_r", name="ones_r")
        dma(out=ones_r, in_=ones_d[:])
        o1128 = persist.tile([1, 128], F32R, tag="o1128", name="o1128")
        dma(out=o1128, in_=o1128_d[:])
        eps_sb = persist.tile([128, 1], F32, tag="eps_sb", name="eps_sb")
        nc.vector.memset(eps_sb, EPS)
        ones1_bf = persist.tile([1, 128], BF16, tag="o1bf", name="o1bf")
        nc.vector.memset(ones1_bf, 1.0)

        # ---- MM1a: qkT [1024, L] = silu(wqk.T @ hT + bqk) ----
        qkT = [persist.tile([128, L], BF16, tag=f"qkT{j}", name=f"qkT{j}") for j in range(8)]
        for j in (0, 4, 1, 5, 2, 6, 3, 7):
            ps = ps_mm.tile([128, 2, 512], F32, tag="mm", name="mm1a")
            for c in range(2):
                for k in range(4):
                    nc.tensor.matmul(
                        ps[:, c, :],
                        wqk[k][:, 128 * j : 128 * (j + 1)],
                        hT[k][:, 512 * c : 512 * (c + 1)],
                        start=(k == 0),
                        stop=(k == 3),
                    )
            nc.scalar.activation(
                out=qkT[j], in_=ps.rearrange("p a b -> p (a b)"),
                func=AF.Silu, bias=bqk[j],
            )

        # ---- MM1b: v = silu(h @ wv + bv), packed into v_aug with ones col ----
        v_aug = [persist.tile([128, 8, 65], BF16, tag=f"vaug{t}", name=f"vaug{t}") for t in range(8)]
        for t in range(8):
            nc.vector.memset(v_aug[t][:, :, 64:65], 1.0)
        for tp in range(4):
            ps = ps_mm.tile([128, 2, 512], F32, tag="mm", name="mm1b")
            for i in range(2):
                t = 2 * tp + i
                for k in range(4):
                    nc.tensor.matmul(
                        ps[:, i, :],
                        hT[k][:, 128 * t : 128 * (t + 1)],
                        wv[k],
                        start=(k == 0),
                        stop=False,
                    )
                nc.tensor.matmul(
                    ps[:, i, :], ones1_bf, bv_bf, start=False, stop=True,
                )
                nc.scalar.activation(
                    out=v_aug[t][:, :, 0:64],
                    in_=ps[:, i, :].rearrange("p (h d) -> p h d", d=64),
                    func=AF.Silu,
                )

        # ---- q/k rms factors: batched mean-square + one Ln/Exp rsqrt ----
        msqs = ps_mm.tile([32, 2, 512], F32, tag="mm", name="msqs")
        for c in range(2):
            for j in range(8):
                sq = sqp.tile([128, 512], BF16, tag="sq", name="sq")
                nc.gpsimd.tensor_mul(
                    out=sq,
                    in0=qkT[j][:, 512 * c : 512 * (c + 1)],
                    in1=qkT[j][:, 512 * c : 512 * (c + 1)],
                )
                nc.tensor.matmul(
                    msqs[:, c, :], e4all[:, j, :], sq,
                    start=(j == 0), stop=(j == 7),
                )
        lnm = persist.tile([32, L], F32, tag="lnm", name="lnm")
        nc.scalar.activation(
            out=lnm, in_=msqs.rearrange("p a b -> p (a b)"),
            func=AF.Ln, scale=1.0 / 32.0, bias=eps_sb[0:32],
        )
        rall = persist.tile([32, L], F32R, tag="rall", name="rall")
        nc.scalar.activation(out=rall, in_=lnm, func=AF.Exp, scale=-0.5)
        for j in range(8):
            rbc = ps_mm.tile([128, 2, 512], F32, tag="mm", name="rbc")
            for c in range(2):
                nc.tensor.matmul(
                    rbc[:, c, :], b4all[:, j, :],
                    rall[:, 512 * c : 512 * (c + 1)],
                    start=True, stop=True,
                )
            for c in range(2):
                if j < 4:
                    nc.vector.scalar_tensor_tensor(
                        out=qkT[j][:, 512 * c : 512 * (c + 1)],
                        in0=qkT[j][:, 512 * c : 512 * (c + 1)],
                        scalar=qmul[j],
                        in1=rbc[:, c, :],
                        op0=OP.mult,
                        op1=OP.mult,
                    )
                else:
                    nc.vector.tensor_mul(
                        out=qkT[j][:, 512 * c : 512 * (c + 1)],
                        in0=qkT[j][:, 512 * c : 512 * (c + 1)],
                        in1=rbc[:, c, :],
                    )

        # ---- attention ----
        diffn = [persist.tile([128, L], F32R, tag=f"diffn{j}", name=f"diffn{j}") for j in range(4)]
        msq2_all = persist.tile([97, 2, L], F32, tag="msq2", name="msq2")
        nc.vector.memset(msq2_all, 1.0)
        for h in range(H):
            jq, jk = h // 2, 4 + h // 2
            po = 64 * (h % 2)
            for c in range(2):
                nt = 4 * c + 4
                accs = [
                    ps_acc.tile([97, 512], F32, tag="acc", name="acc")
                    for _ in range(2)
                ]
                for t in range(nt):
                    off = max(0, 128 * (t - 4 * c))
                    w = 512 - off
                    diag = t >= 4 * c
                    sc = ps_mm.tile([128, 2, 512], F32, tag="mm", name="sc")
                    for br in range(2):
                        bo = po + 32 * br
                        nc.tensor.matmul(
                            sc[:, br, 0:w],
                            qkT[jk][bo : bo + 32, 128 * t : 128 * (t + 1)],
                            qkT[jq][bo : bo + 32, 512 * c + off : 512 * (c + 1)],
                            start=True,
                            stop=not diag,
                            tile_position=(bo, 0),
                        )
                    if diag:
                        for br in range(2):
                            nc.tensor.matmul(
                                sc[:, br, 0:128],
                                triu_sb,
                                neye_sb,
                                start=False,
                                stop=True,
                            )
                    p = p_pool.tile([128, 2, 512], BF16, tag="p", name="p")
                    nc.scalar.activation(
                        out=p[:, :, 0:w], in_=sc[:, :, 0:w], func=AF.Exp, scale=SCALE
                    )
                    for br in range(2):
                        nc.tensor.matmul(
                            accs[br][0:65, off:512],
                            v_aug[t][:, h, :],
                            p[:, br, 0:w],
                            start=(t == 0),
                            stop=(t == nt - 1),
                        )
                # u_whs = whs*(o1*bc(den2) - lam*o2*bc(den1)); the den1*den2
                # scale cancels in the rms. whs and lam*whs ride the broadcast
                # matmul stationaries; the rms mean-square divides whs^2 back
                # out via the inv-whs^2 dmsq stationary.
                osb = []
                for br in range(2):
                    o = osbp.tile([65, 512], F32R, tag="osb", name="osb")
                    nc.vector.tensor_copy(out=o, in_=accs[br][0:65, :])
                    osb.append(o)
                bbt = ps_mm.tile([128, 2, 512], F32, tag="mm", name="bbc")
                nc.tensor.matmul(
                    bbt[0:64, 0, :], lamwhs_t[64:65, :], osb[0][64:65, :],
                    start=True, stop=True,
                )
                nc.tensor.matmul(
                    bbt[0:64, 1, :], whs64_t[64:65, :], osb[1][64:65, :],
                    start=True, stop=True,
                )
                m1 = m1p.tile([64, 512], F32, tag="m1", name="m1")
                nc.vector.tensor_mul(out=m1, in0=osb[1][0:64, :], in1=bbt[0:64, 0, :])
                t2 = t2p.tile([64, 512], F32, tag="t2", name="t2")
                nc.vector.tensor_mul(out=t2, in0=osb[0][0:64, :], in1=bbt[0:64, 1, :])
                uslice = diffn[jq][po : po + 64, 512 * c : 512 * (c + 1)]
                nc.vector.tensor_sub(out=uslice, in0=t2, in1=m1)
                usq = usqp.tile([64, 512], BF16, tag="usq", name="usq")
                nc.gpsimd.tensor_mul(out=usq, in0=uslice, in1=uslice)
                nc.tensor.matmul(
                    accs[0][96:97, :], invwhsq, usq,
                    start=True, stop=True, tile_position=(0, 96),
                )
                nc.vector.tensor_copy(
                    out=msq2_all[
                        32 * (h % 4) : 32 * (h % 4) + 1,
                        h // 4,
                        512 * c : 512 * (c + 1),
                    ],
                    in_=accs[0][96:97, :],
                )

        # ---- diff rms: one batched Ln/Exp + broadcast + apply ----
        lnm2 = persist.tile([97, 2, L], F32, tag="lnm2", name="lnm2")
        nc.scalar.activation(
            out=lnm2, in_=msq2_all.rearrange("p a b -> p (a b)"),
            func=AF.Ln, scale=1.0 / 64.0, bias=eps_sb[0:97],
        )
        rt2 = persist.tile([97, 2, L], F32R, tag="rt2", name="rt2")
        nc.scalar.activation(
            out=rt2, in_=lnm2.rearrange("p a b -> p (a b)"), func=AF.Exp, scale=-0.5
        )
        for j in range(4):
            rbc2 = ps_mm.tile([128, 2, 512], F32, tag="mm", name="rbc2")
            for c in range(2):
                nc.tensor.matmul(
                    rbc2[:, c, :], selall[:, j % 2, :],
                    rt2[:, j // 2, 512 * c : 512 * (c + 1)],
                    start=True, stop=True,
                )
            for c in range(2):
                nc.vector.tensor_mul(
                    out=diffn[j][:, 512 * c : 512 * (c + 1)],
                    in0=diffn[j][:, 512 * c : 512 * (c + 1)],
                    in1=rbc2[:, c, :],
                )

        # ---- MM2: attn = silu(wout.T @ diffn + bout) ----
        attn = [persist.tile([128, L], F32R, tag=f"attn{j}", name=f"attn{j}") for j in range(4)]
        for j in range(4):
            ps = ps_mm.tile([128, 2, 512], F32, tag="mm", name="mm2")
            for c in range(2):
                for k in range(4):
                    nc.tensor.matmul(
                        ps[:, c, :],
                        wout_sb[k][:, 128 * j : 128 * (j + 1)],
                        diffn[k][:, 512 * c : 512 * (c + 1)],
                        start=(k == 0),
                        stop=(k == 3),
                    )
            nc.scalar.activation(
                out=attn[j], in_=ps.rearrange("p a b -> p (a b)"),
                func=AF.Silu, bias=bout[j],
            )

        # ---- final rms over CW=512 + residual + routing weight ----
        fin = ps_mm.tile([1, 2, 512], F32, tag="mm", name="fin")
        for c in range(2):
            for j in range(4):
                asq = asqp.tile([128, 512], F32R, tag="asq", name="asq")
                nc.gpsimd.tensor_mul(
                    out=asq,
                    in0=attn[j][:, 512 * c : 512 * (c + 1)],
                    in1=attn[j][:, 512 * c : 512 * (c + 1)],
                )
                nc.tensor.matmul(
                    fin[:, c, :], ones_r, asq, start=(j == 0), stop=(j == 3)
                )
        lnf = persist.tile([1, L], F32, tag="lnf", name="lnf")
        nc.scalar.activation(
            out=lnf, in_=fin.rearrange("p a b -> p (a b)"),
            func=AF.Ln, scale=1.0 / 512.0, bias=eps_sb[0:1],
        )
        rf = persist.tile([1, L], F32R, tag="rf", name="rf")
        nc.scalar.activation(out=rf, in_=lnf, func=AF.Exp, scale=-0.5)
        for j in range(4):
            nc.vector.tensor_scalar_mul(out=hT[j], in0=hT[j], scalar1=wrt)
        rfbc = ps_mm.tile([128, 2, 512], F32, tag="mm", name="rfbc")
        for c in range(2):
            nc.tensor.matmul(
                rfbc[:, c, :], o1128, rf[:, 512 * c : 512 * (c + 1)],
                start=True, stop=True,
            )
        for c in range(2):
            for j in range(4):
                tmp = tmpp.tile([128, 512], F32, tag="tmp", name="tmp")
                nc.vector.tensor_mul(
                    out=tmp, in0=attn[j][:, 512 * c : 512 * (c + 1)],
                    in1=rfbc[:, c, :],
                )
                y = yp.tile([128, 512], F32, tag="y", name="y")
                nc.vector.scalar_tensor_tensor(
                    out=y,
                    in0=tmp,
                    scalar=wnw[j],
                    in1=hT[j][:, 512 * c : 512 * (c + 1)],
                    op0=OP.mult,
                    op1=OP.add,
                )
                dma2(out=yT_d[128 * j : 128 * (j + 1), 512 * c : 512 * (c + 1)], in_=y)
        est.close()

    nc.compile()
    return nc


def kernel(x, routing_weights, Wqkv, bqkv, Wout, bout, lq1, lk1, lq2, lk2, wq, wk, wh, wn):
    if "nc" not in _cache:
        _cache["nc"] = _build()
    nc = _cache["nc"]

    x = np.asarray(x, np.float32)
    routing_weights = np.asarray(routing_weights, np.float32)
    Wqkv = np.asarray(Wqkv, np.float32)
    bqkv = np.asarray(bqkv, np.float32)
    Wout = np.asarray(Wout, np.float32)
    bout = np.asarray(bout, np.float32)
    lq1, lk1 = np.asarray(lq1, np.float32), np.asarray(lk1, np.float32)
    lq2, lk2 = np.asarray(lq2, np.float32), np.asarray(lk2, np.float32)
    wq, wk = np.asarray(wq, np.float32), np.asarray(wk, np.float32)
    wh, wn = np.asarray(wh, np.float32), np.asarray(wn, np.float32)

    triu = np.triu(np.ones((128, 128), np.float32), 1).astype(ml_dtypes.bfloat16)
    neye = (MASKC * np.eye(128, dtype=np.float32)).astype(ml_dtypes.bfloat16)
    e4all = np.zeros((128, 8, 32), np.float32)
    for j in range(8):
        for g in range(4):
            e4all[32 * g : 32 * (g + 1), j, 4 * j + g] = 1.0
    e4all = e4all.astype(ml_dtypes.bfloat16)
    b4all = np.zeros((32, 8, 128), np.float32)
    for j in range(8):
        for g in range(4):
            b4all[4 * j + g, j, 32 * g : 32 * (g + 1)] = 1.0
    selall = np.zeros((97, 2, 128), np.float32)
    for p in range(128):
        selall[32 * (p // 64), 0, p] = 1.0        # j even: heads h%4 in {0,1}
        selall[32 * (2 + p // 64), 1, p] = 1.0    # j odd:  heads h%4 in {2,3}
    ones = np.ones((128, 1), np.float32)
    o1128 = np.ones((1, 128), np.float32)

    in_maps = []
    for i in range(8):
        b, n = i // 4, i % 4
        w = float(routing_weights[b, n])
        lam = float(
            np.exp(np.dot(lq1[n], lk1[n]).astype(np.float32))
            - np.exp(np.dot(lq2[n], lk2[n]).astype(np.float32))
            + np.float32(LAM0)
        )
        wqwk = (wq[n] * wk[n]).astype(np.float32)  # [32]
        in_maps.append(
            dict(
                hT=np.ascontiguousarray(x[b, :, CW * n : CW * (n + 1)].T),
                wqk=np.ascontiguousarray(Wqkv[n][:, : 2 * CW]),
                wv=np.ascontiguousarray(Wqkv[n][:, 2 * CW :]),
                wout=np.ascontiguousarray(Wout[n]),
                bqk=np.ascontiguousarray(bqkv[n][: 2 * CW].reshape(8, 128, 1)),
                bv=np.ascontiguousarray(
                    bqkv[n][2 * CW :].reshape(1, CW)
                ).astype(ml_dtypes.bfloat16),
                bout=np.ascontiguousarray(bout[n].reshape(4, 128, 1)),
                qmul=np.ascontiguousarray(np.tile(wqwk, 16).reshape(4, 128, 1)),
                lamwhs=np.ascontiguousarray((wh[n] * (0.8 * lam)).reshape(1, 64)).astype(np.float32),
                whs64=np.ascontiguousarray((wh[n] * 0.8).reshape(1, 64)).astype(np.float32),
                invwhsq=np.ascontiguousarray(
                    (1.0 / (wh[n] * 0.8) ** 2).reshape(64, 1)
                ).astype(ml_dtypes.bfloat16),
                wnw=np.ascontiguousarray((wn[n] * w).reshape(4, 128, 1)).astype(np.float32),
                wrt=np.full((128, 1), w, np.float32),
                triu=triu,
                neye=neye,
                e4all=e4all,
                b4all=b4all,
                selall=selall,
                ones=ones,
                o1128=o1128,
            )
        )

    prof_dir = os.environ.get("KERNEL_PROFILE_DIR")
    if prof_dir:
        res = run_bass_kernel_spmd(
            nc, in_maps, list(range(8)), trace=True, tmpdir=prof_dir
        )
        _cache["exec_time_ns"] = res.exec_time_ns
    else:
        res = run_bass_kernel_spmd(nc, in_maps, list(range(8)))

    out = np.empty((B, L, N_CH * CW), np.float32)
    for i in range(8):
        b, n = i // 4, i % 4
        out[b, :, CW * n : CW * (n + 1)] = res.results[i]["yT"].T
    return out
